# revision 22
# baseline (speedup 1.0000x reference)
"""Haar DWT decoder (2-level inverse, zero details) as a Trainium2 Bass kernel.

out[b, c, j, k] = z[b].reshape(C, 128, 128)[c, j//4, k//4] * 0.25
i.e. a 4x4 nearest-neighbor upsample scaled by 1/4.

Data-parallel over batch: 128 samples -> 16 per core on 8 NeuronCores.

Per-core shape of the problem: read 3 MiB of z, write 48 MiB of output
through 16 SDMA engines at ~26.5 GB/s each (~424 GB/s aggregate), so the
floor is ~122 us of streaming plus the pipeline lead-in.

All DMA (loads and stores) goes through the two HWDGE rings (sync +
scalar). SWDGE (gpsimd) is deliberately unused: its descriptor rings
live on SBUF partitions whose AXI ports are shared with SDMA engines
7/15 (a known straggler cause), and the SWDGE datapath is ~30% slower
per packet. All input loads are issued before the first store — six
individual DMAs plus one batched DMA for the remaining ten samples —
so their ~8.7 us of engine work fills the ramp window between the
preamble barrier and the first store packets; in the clean-run trace
the 16 SDMA engines are then busy without a gap from ~9.6 us until the
last store packet at ~136 us (measured 100% utilization).
"""

import numpy as np

import concourse.bass as bass
import concourse.mybir as mybir
import concourse.tile as tile
from concourse.bass_utils import run_bass_kernel_spmd

# The walrus build in this container rejects instructions carrying more than
# one sync-wait command (codegen: "Too many sync wait commands" — observed on
# a Drain with 3 waits and a DMACopy with 2). Tile freely attaches several
# waits to one instruction, so after tracing we split the excess onto NOPs
# inserted just before the instruction on the same engine; sequential
# dispatch on one engine makes that equivalent.
_MAX_WAITS = 1


def _split_excess_waits(nc: bass.Bass) -> None:
    for f in nc.m.functions:
        for bb in f.blocks:
            insns = bb.instructions
            # Iterate over a snapshot; mutate the live list via insert.
            for ins in list(insns):
                si = ins.sync_info
                if si is None or not si.on_wait or len(si.on_wait) <= _MAX_WAITS:
                    continue
                waits = list(si.on_wait)
                keep = waits[-_MAX_WAITS:]
                spill = waits[:-_MAX_WAITS]
                pos = insns.index(ins)
                nops = []
                for i in range(0, len(spill), _MAX_WAITS):
                    nop = nc.engines[ins.engine].nop(nofuse=True).ins
                    # nop() appended itself to the current bb; pull it out.
                    cur = nc.cur_bb.bb.instructions
                    assert cur[-1] is nop
                    cur.pop()
                    nop.sync_info = mybir.SyncInfo(
                        on_wait=spill[i : i + _MAX_WAITS], on_update=[]
                    )
                    nops.append(nop)
                insns[pos:pos] = nops
                ins.sync_info = mybir.SyncInfo(
                    on_wait=keep, on_update=list(si.on_update)
                )

# Problem constants (hardcoded: module config out_shape=(3,512,512), levels=2)
BATCH = 128
C = 3
CAH = 128  # coarse-approximation spatial dims
CAW = 128
S = 4      # 2**levels upsample factor
H = 512
W = 512
N_CORES = 8
B_SHARD = BATCH // N_CORES  # 16

# Six individual preloads keep the first store's ring position early; the
# remaining ten samples load as ONE big DMA on ring B (single trigger), so
# all 3 MiB of load traffic (~7.7 us of engine work) lands in the
# otherwise-idle engine window between the preamble barrier (~8.6 us) and
# the first store packets (~16.2 us), without delaying S0. Steady state is
# then a pure 24 KiB-packet store stream.
PRELOAD = 6

F32 = mybir.dt.float32


def _build_nc(b_shard: int = B_SHARD) -> bass.Bass:
    nc = bass.Bass("TRN2", target_bir_lowering=False, debug=False)
    z = nc.dram_tensor("z", [b_shard, C * CAH * CAW], F32, kind="ExternalInput").ap()
    # Output is declared FLAT per sample and reshaped to (C, H, W) in numpy:
    # a coarse row r = c*128+jc owns exactly the 2048 contiguous output
    # floats at offset 2048*r, so partition p holding rows 3p..3p+2 stores a
    # fully-contiguous 24 KiB run — 3x bigger descriptors than the
    # channel-major layout, and the load becomes perfectly contiguous too
    # (1536 B runs instead of the transpose layout's 512 B).
    out = nc.dram_tensor("out", [b_shard, C * H * W], F32, kind="ExternalOutput").ap()

    def ring(i: int):
        return nc.sync if i % 2 == 0 else nc.scalar

    with tile.TileContext(nc) as tc:
        with (
            tc.tile_pool(name="zin", bufs=PRELOAD + 1) as zin_pool,
            tc.tile_pool(name="ztail", bufs=1) as zt_pool,
            tc.tile_pool(name="wide", bufs=6) as w_pool,
        ):
            zts: list = []

            def issue_load(b: int) -> None:
                # Fully-contiguous load: partition p gets z[b][384p:384p+384]
                # (= coarse rows 3p..3p+2 in (c*128+jc) order).
                zt = zin_pool.tile([CAH, 3 * CAW], F32)
                zts.append(zt)
                ring(b).dma_start(
                    out=zt[:], in_=z[b].rearrange("(p x) -> p x", p=CAH)
                )

            for b in range(PRELOAD):
                issue_load(b)

            # Samples PRELOAD..15 in one DMA on ring B (scalar): one 0.7 us
            # trigger instead of ten, and its 1.9 MiB drains during the ramp.
            rest = b_shard - PRELOAD
            ztail = zt_pool.tile([CAH, rest * 3 * CAW], F32)
            nc.scalar.dma_start(
                out=ztail[:].rearrange("p (b x) -> p b x", b=rest),
                in_=z[PRELOAD:].rearrange("b (p x) -> p b x", p=CAH),
            )
            for b in range(PRELOAD, b_shard):
                zts.append(ztail[:, (b - PRELOAD) * 3 * CAW : (b - PRELOAD + 1) * 3 * CAW])

            for b in range(b_shard):
                zt = zts[b]
                ztv = zt[:] if hasattr(zt, "tag_meta") else zt
                zq = ztv.rearrange("p (q kc) -> p q kc", q=3)

                # Partition p materializes its 3 coarse rows' upsampled
                # output: free layout (q, jr, kc, kr), 24 KiB per partition,
                # which IS the flat output byte range [24KiB*p, 24KiB*(p+1)).
                w2 = w_pool.tile([CAH, 3 * S * W], F32, tag="wide")
                w2v = w2[:].rearrange(
                    "p (q jr kc kr) -> p q jr kc kr", q=3, jr=S, kc=CAW, kr=S
                )
                w2f = w2[:].rearrange("p (q jr k) -> p q jr k", q=3, jr=S)

                # Width-expand x4 (with the 1/4 scale) via a 0-stride
                # broadcast input; height-replicate jr=1..3 split across DVE
                # and ACT (gpsimd's tensor_copy is ~4x slower — don't).
                zb = zq.unsqueeze(3).broadcast_to([CAH, 3, CAW, S])
                nc.vector.tensor_scalar_mul(w2v[:, :, 0, :, :], zb, 0.25)
                nc.scalar.copy(w2f[:, :, 1, :], w2f[:, :, 0, :])
                nc.vector.tensor_copy(w2f[:, :, 2, :], w2f[:, :, 0, :])
                nc.scalar.copy(w2f[:, :, 3, :], w2f[:, :, 0, :])

                # One fully-contiguous 3 MiB store per sample, 24 KiB
                # descriptor runs on both sides; alternate HWDGE rings.
                ring(b).dma_start(
                    out=out[b].rearrange("(p x) -> p x", p=CAH), in_=w2[:]
                )


    _split_excess_waits(nc)
    return nc


_NC_CACHE: dict[int, bass.Bass] = {}


def _get_nc(b_shard: int = B_SHARD) -> bass.Bass:
    if b_shard not in _NC_CACHE:
        _NC_CACHE[b_shard] = _build_nc(b_shard)
    return _NC_CACHE[b_shard]


def kernel(z: np.ndarray) -> np.ndarray:
    z = np.ascontiguousarray(z, dtype=np.float32)
    assert z.shape == (BATCH, C * CAH * CAW), z.shape
    nc = _get_nc()
    in_maps = [
        {"z": z[i * B_SHARD : (i + 1) * B_SHARD]} for i in range(N_CORES)
    ]
    res = run_bass_kernel_spmd(nc, in_maps, list(range(N_CORES)))
    return np.concatenate(
        [res.results[i]["out"].reshape(B_SHARD, C, H, W) for i in range(N_CORES)],
        axis=0,
    )
